# revision 10
# baseline (speedup 1.0000x reference)
"""Trainium2 Bass kernel: causal attention with 3D (Rodrigues) RoPE.

Sharding: tensor-parallel over heads (2 heads/core on 8 cores) for
QKV projection + RoPE + SDPA, then an AllToAll redistributes attention
outputs so the output projection is sharded over tokens (512/core).
The A2A is split per local head so the first half overlaps attention,
and the output projection's contraction is split to match.

Layouts (per core, all matmuls in float32r):
  x^T       [1536, 4096]   tokens on the free axis
  qkv proj  computed transposed: out rows [q0 q1 k0 k1 v0 v1] (576) in
                           5 M-tiles of N=512; straddled tiles evicted
                           with 32-aligned cross-base copies on ScalarE
  q^T,k^T   [96, 4096]     head-dim on partitions (plane-major triplet
                           order so RoPE shifts are 32-row blocks)
  V         [4096, 194]    built by PE transposes of v^T; cols =
                           v_h0|1|v_h1|1 (the 1-columns give the softmax
                           denominator as row 96 of the PV matmul)
  S^T       [tk=128, tq=512] softmax's reduction axis = PE contraction
                           axis -> no transposes in attention.
"""

import sys

sys.path.insert(0, "/opt/trn_rl_repo")

import numpy as np

D_MODEL, N_HEADS, HEAD_DIM, MAX_POS = 1536, 16, 96, 4096
B, T = 2, 2048
NTOK = B * T                      # 4096
NCORES = 8
HPC = N_HEADS // NCORES           # 2 heads per core
NTRIP = HEAD_DIM // 3             # 32 triplets
KT = D_MODEL // 128               # 12 contraction tiles
NCH = NTOK // 512                 # 8 token chunks of 512
TQC = T // 512                    # 4 query chunks per batch
SCALE = 1.0 / np.sqrt(HEAD_DIM)

_CACHE = {}


def _build_nc():
    import concourse.bass as bass
    import concourse.mybir as mybir
    import concourse.tile as tile
    from concourse import bacc

    f32 = mybir.dt.float32
    f32r = mybir.dt.float32r
    MUL = mybir.AluOpType.mult
    ADD = mybir.AluOpType.add

    nc = bacc.Bacc("TRN2", target_bir_lowering=False, debug=False,
                   enable_asserts=False, num_devices=NCORES)

    xT = nc.dram_tensor("xT", [D_MODEL, NTOK], f32r, kind="ExternalInput").ap()
    wallT = nc.dram_tensor("wallT", [D_MODEL, 576], f32r, kind="ExternalInput").ap()
    woT = nc.dram_tensor("woT", [D_MODEL, D_MODEL], f32r, kind="ExternalInput").ap()
    cco = nc.dram_tensor("cco", [96, 3, T], f32, kind="ExternalInput").ap()
    msk = nc.dram_tensor("msk", [128, 4, 512], f32r, kind="ExternalInput").ap()
    idn = nc.dram_tensor("idn", [96, 96], f32r, kind="ExternalInput").ap()
    out = nc.dram_tensor("out", [D_MODEL, 512], f32, kind="ExternalOutput").ap()

    with tile.TileContext(nc) as tc:
        with tc.tile_pool(name="dram", bufs=1, space="DRAM") as dram:
            a2a_in = [dram.tile([NCH, 96, 512], f32r, name=f"a2a_in{h}")
                      for h in range(HPC)]
            a2a_out = [dram.tile([NCH, 96, 512], f32r, name=f"a2a_out{h}")
                       for h in range(HPC)]

            with tc.tile_pool(name="ph12", bufs=1) as pp:
                qk_rot = [pp.tile([96, NTOK], f32r, tag=f"qkrot{i}",
                                  name=f"qkrot{i}") for i in range(4)]
                v_sb = pp.tile([128, NTOK // 128, 194], f32r, tag="vsb")

                # ------------ phase 1: qkv projection + rope ------------
                with tc.tile_pool(name="ph1", bufs=1) as p1, \
                     tc.tile_pool(name="ph1s", bufs=2) as p1s, \
                     tc.tile_pool(name="ps_qk", bufs=6, space="PSUM") as ps_qk, \
                     tc.tile_pool(name="ps_t", bufs=2, space="PSUM") as ps_t:
                    wall_sb = p1.tile([128, KT, 576], f32r, tag="wall")
                    nc.sync.dma_start(
                        wall_sb[:], wallT.rearrange("(k p) c -> p k c", p=128))
                    ident = p1.tile([96, 96], f32r, tag="ident")
                    nc.sync.dma_start(ident[:], idn[:])

                    for ch in range(NCH):
                        coff = (ch % TQC) * 512   # position within batch
                        c_sl = p1s.tile([96, 3, 512], f32, tag="csl")
                        nc.sync.dma_start(c_sl[:], cco[:, :, coff:coff + 512])
                        xt = []
                        for kt in range(KT):
                            t = p1s.tile([128, 512], f32r, tag=f"xt{kt}",
                                         name=f"xt{kt}")
                            nc.sync.dma_start(
                                t[:], xT[kt * 128:(kt + 1) * 128,
                                         ch * 512:(ch + 1) * 512])
                            xt.append(t)
                        # 5 M-tiles over rows [q0 q1 k0 k1 v0 v1] (576)
                        pst = []
                        for m in range(5):
                            mw = 128 if m < 4 else 64
                            ps = ps_qk.tile([128, 512], f32, tag="qk",
                                            name="ps")
                            for kt in range(KT):
                                nc.tensor.matmul(
                                    ps[0:mw, :],
                                    wall_sb[:, kt, m * 128:m * 128 + mw],
                                    xt[kt][:], start=(kt == 0),
                                    stop=(kt == KT - 1))
                            pst.append(ps)
                        # straddled eviction (ScalarE, 32-aligned pieces)
                        raws = []
                        for nm in ("q0", "q1", "k0", "k1"):
                            r = p1s.tile([96, 512], f32, tag=f"raw{nm}",
                                         name=f"raw{nm}")
                            raws.append(r)
                        vr = []
                        for hh in range(HPC):
                            r = p1s.tile([96, 512], f32r, tag=f"vr{hh}",
                                         name=f"vr{hh}")
                            vr.append(r)
                        CP = mybir.ActivationFunctionType.Copy
                        ev = [
                            (raws[0][0:96, :], pst[0][0:96, :]),
                            (raws[1][0:32, :], pst[0][96:128, :]),
                            (raws[1][32:64, :], pst[1][0:32, :]),
                            (raws[1][64:96, :], pst[1][32:64, :]),
                            (raws[2][0:64, :], pst[1][64:128, :]),
                            (raws[2][64:96, :], pst[2][0:32, :]),
                            (raws[3][0:32, :], pst[2][32:64, :]),
                            (raws[3][32:64, :], pst[2][64:96, :]),
                            (raws[3][64:96, :], pst[2][96:128, :]),
                            (vr[0][0:96, :], pst[3][0:96, :]),
                            (vr[1][0:32, :], pst[3][96:128, :]),
                            (vr[1][32:64, :], pst[4][0:32, :]),
                            (vr[1][64:96, :], pst[4][32:64, :]),
                        ]
                        for dst_ap, src_ap in ev:
                            nc.scalar.activation(dst_ap, src_ap, CP)
                        # rope on q0,q1,k0,k1
                        for m in range(4):
                            raw = raws[m]
                            dst = qk_rot[m][:, ch * 512:(ch + 1) * 512]
                            nc.vector.tensor_tensor(
                                dst, raw[:], c_sl[:, 0, :], MUL)
                            g1 = p1s.tile([96, 512], f32, tag="g1")
                            nc.sync.dma_start(g1[0:64, :], raw[32:96, :])
                            nc.sync.dma_start(g1[64:96, :], raw[0:32, :])
                            g2 = p1s.tile([96, 512], f32, tag="g2")
                            nc.sync.dma_start(g2[0:32, :], raw[64:96, :])
                            nc.sync.dma_start(g2[32:96, :], raw[0:64, :])
                            nc.vector.tensor_tensor(
                                g1[:], g1[:], c_sl[:, 1, :], MUL)
                            nc.vector.tensor_tensor(dst, dst, g1[:], ADD)
                            nc.vector.tensor_tensor(
                                g2[:], g2[:], c_sl[:, 2, :], MUL)
                            nc.vector.tensor_tensor(dst, dst, g2[:], ADD)
                        # V: PE-transpose v^T -> [tokens, dims]
                        for hh in range(HPC):
                            for ts_ in range(4):
                                pt_ = ps_t.tile([128, 96], f32r, tag="tr",
                                                name="pt_")
                                nc.tensor.transpose(
                                    pt_[:],
                                    vr[hh][:, ts_ * 128:(ts_ + 1) * 128],
                                    ident[:])
                                g = ch * 4 + ts_
                                nc.scalar.activation(
                                    v_sb[:, g, hh * 97:hh * 97 + 96],
                                    pt_[:], CP)
                                nc.vector.memset(
                                    v_sb[:, g, hh * 97 + 96:hh * 97 + 97]
                                    .bitcast(f32), 1.0)

                # ------------ phase 2: attention (+ overlapped A2A/o-proj) --
                with tc.tile_pool(name="ph2", bufs=6) as p2, \
                     tc.tile_pool(name="ph2b", bufs=2) as p2b, \
                     tc.tile_pool(name="ph2c", bufs=1) as p2c, \
                     tc.tile_pool(name="ps_s", bufs=3, space="PSUM") as ps_s, \
                     tc.tile_pool(name="ps_pv", bufs=2, space="PSUM") as ps_pv, \
                     tc.tile_pool(name="ps_o", bufs=3, space="PSUM") as ps_o:
                    m_sb = p2c.tile([128, 4, 512], f32r, tag="msb")
                    nc.sync.dma_start(m_sb[:], msk[:])

                    def attention(h):
                        for b in range(B):
                            for cl in range(TQC):
                                qoff = b * T + cl * 512
                                pv = ps_pv.tile([128, 512], f32, tag="pv",
                                                name="pv")
                                ntk = 4 * cl + 4
                                for tt in range(ntk):
                                    koff = b * T + tt * 128
                                    sp = ps_s.tile([128, 512], f32, tag="s",
                                                   name="sp")
                                    nc.tensor.matmul(
                                        sp[:],
                                        qk_rot[2 + h][:, koff:koff + 128],
                                        qk_rot[h][:, qoff:qoff + 512],
                                        start=True, stop=True)
                                    pt = p2.tile([128, 512], f32r, tag="p",
                                                 name="pt")
                                    nc.scalar.activation(
                                        pt[:], sp[:],
                                        mybir.ActivationFunctionType.Exp)
                                    if tt >= 4 * cl:
                                        nc.gpsimd.tensor_tensor(
                                            pt[:], pt[:],
                                            m_sb[:, tt - 4 * cl, :], MUL)
                                    nc.tensor.matmul(
                                        pv[0:97, :],
                                        v_sb[:, b * 16 + tt,
                                             h * 97:h * 97 + 97],
                                        pt[:], start=(tt == 0),
                                        stop=(tt == ntk - 1))
                                linv = p2b.tile([1, 512], f32, tag="linv",
                                                name="linv")
                                nc.vector.reciprocal(linv[:], pv[96:97, :])
                                brow = p2b.tile([96, 512], f32, tag="brow",
                                                name="brow")
                                nc.gpsimd.partition_broadcast(brow[:], linv[:])
                                att = p2b.tile([96, 512], f32r, tag="att",
                                               name="att")
                                nc.vector.tensor_tensor(
                                    att[:], pv[0:96, :], brow[:], MUL)
                                nc.sync.dma_start(
                                    a2a_in[h][b * TQC + cl, :, :], att[:])

                    def oproj_half(half, partA_sb):
                        """o-proj contraction over one head-half (6 K-tiles)."""
                        flat = a2a_out[half][:].rearrange("a b c -> (a b) c")
                        att2 = []
                        for et in range(6):
                            t = p2c.tile([128, 512], f32r,
                                         tag=f"att2_{half}_{et}",
                                         name=f"att2_{half}_{et}")
                            nc.sync.dma_start(
                                t[:], flat[et * 128:(et + 1) * 128, :])
                            att2.append(t)
                        for g4 in range(4):          # 4 groups of 3 dt tiles
                            pos = [ps_o.tile([128, 512], f32, tag="o",
                                             name=f"po_{half}_{g4}_{i}")
                                   for i in range(3)]
                            for et in range(6):
                                wot = p2c.tile(
                                    [128, 384], f32r, tag="wos", bufs=4,
                                    name="wot")
                                nc.sync.dma_start(
                                    wot[:],
                                    woT[half * 768 + et * 128:
                                        half * 768 + (et + 1) * 128,
                                        g4 * 384:(g4 + 1) * 384])
                                for i in range(3):
                                    nc.tensor.matmul(
                                        pos[i][:],
                                        wot[:, i * 128:(i + 1) * 128],
                                        att2[et][:], start=(et == 0),
                                        stop=(et == 5),
                                        skip_group_check=True)
                            for i in range(3):
                                dt_ = g4 * 3 + i
                                if half == 0:
                                    nc.vector.tensor_copy(
                                        partA_sb[:, dt_, :], pos[i][:])
                                else:
                                    ot = p2b.tile([128, 512], f32, tag="ot",
                                                  bufs=3, name="ot")
                                    nc.vector.tensor_tensor(
                                        ot[:], pos[i][:],
                                        partA_sb[:, dt_, :], ADD)
                                    nc.sync.dma_start(
                                        out[dt_ * 128:(dt_ + 1) * 128, :],
                                        ot[:])

                    partA_sb = p2c.tile([128, KT, 512], f32, tag="partA")
                    attention(0)
                    nc.gpsimd.collective_compute(
                        "AllToAll", mybir.AluOpType.bypass,
                        replica_groups=[list(range(NCORES))],
                        ins=[a2a_in[0].opt()], outs=[a2a_out[0].opt()])
                    attention(1)
                    oproj_half(0, partA_sb)
                    nc.gpsimd.collective_compute(
                        "AllToAll", mybir.AluOpType.bypass,
                        replica_groups=[list(range(NCORES))],
                        ins=[a2a_in[1].opt()], outs=[a2a_out[1].opt()])
                    oproj_half(1, partA_sb)

    nc.compile()
    return nc


def _plane_major(w):
    """Reorder head-dim rows 3k+i -> 32i+k (per 96-row head block)."""
    idx = np.empty(96, dtype=np.int64)
    for i in range(3):
        for k in range(NTRIP):
            idx[32 * i + k] = 3 * k + i
    return w[idx]


def _prep_inputs(x, w_qkv, w_o, Rs):
    x = np.asarray(x, dtype=np.float32)
    w_qkv = np.asarray(w_qkv, dtype=np.float32)
    w_o = np.asarray(w_o, dtype=np.float32)
    Rs = np.asarray(Rs, dtype=np.float32)

    xT = np.ascontiguousarray(x.reshape(NTOK, D_MODEL).T)

    # rope coefficients, plane-major rows: C[d, delta, t]
    R = Rs[:T]                                   # (T, 32, 3, 3)
    cco = np.empty((96, 3, T), dtype=np.float32)
    for d in range(3):
        for i in range(3):
            cco[32 * i:32 * i + 32, d, :] = R[:, :, i, (i + d) % 3].T

    # causal masks for the 4 diagonal sub-tiles
    msk = np.empty((128, 4, 512), dtype=np.float32)
    j = np.arange(128)[:, None]
    i = np.arange(512)[None, :]
    for m in range(4):
        msk[:, m, :] = (m * 128 + j <= i).astype(np.float32)

    # w_o columns-for-even-heads first, then odd (matches split A2A halves)
    woT = np.ascontiguousarray(w_o.T)            # rows e = h*96+d
    perm = np.concatenate(
        [np.arange(h * 96, (h + 1) * 96) for h in range(0, 16, 2)] +
        [np.arange(h * 96, (h + 1) * 96) for h in range(1, 16, 2)])
    woTp = np.ascontiguousarray(woT[perm])

    def w_row(s, h):                             # rows of w_qkv for (q/k/v, head)
        base = (s * N_HEADS + h) * HEAD_DIM
        return w_qkv[base:base + HEAD_DIM]

    in_maps = []
    for c in range(NCORES):
        h0, h1 = 2 * c, 2 * c + 1
        wall = np.concatenate([
            _plane_major(w_row(0, h0)) * SCALE,
            _plane_major(w_row(0, h1)) * SCALE,
            _plane_major(w_row(1, h0)),
            _plane_major(w_row(1, h1)),
            w_row(2, h0),
            w_row(2, h1),
        ], axis=0)                               # [576, 1536]
        wallT = np.ascontiguousarray(wall.T)     # [1536, 576]
        in_maps.append({
            "xT": xT, "wallT": wallT, "woT": woTp,
            "cco": cco, "msk": msk, "idn": np.eye(96, dtype=np.float32),
        })
    return in_maps


def kernel(x, w_qkv, w_o, Rs):
    from concourse import bass_utils

    if "nc" not in _CACHE:
        _CACHE["nc"] = _build_nc()
    nc = _CACHE["nc"]
    in_maps = _prep_inputs(x, w_qkv, w_o, Rs)
    res = bass_utils.run_bass_kernel_spmd(
        nc, in_maps, core_ids=list(range(NCORES)))
    full_T = np.concatenate([res.results[c]["out"] for c in range(NCORES)],
                            axis=1)              # [1536, 4096]
    return np.ascontiguousarray(full_T.T).reshape(B, T, D_MODEL)


# revision 11
# speedup vs baseline: 1.1266x; 1.1266x over previous
"""Trainium2 Bass kernel: causal attention with 3D (Rodrigues) RoPE.

Sharding: tensor-parallel over heads (2 heads/core on 8 cores) for
QKV projection + RoPE + SDPA, then an AllToAll redistributes attention
outputs so the output projection is sharded over tokens (512/core).
The A2A is split per local head so the first half overlaps attention,
and the output projection's contraction is split to match.

Layouts (per core, all matmuls in float32r):
  x^T       [1536, 4096]   tokens on the free axis
  qkv proj  computed transposed: out rows [q0 q1 k0 k1 v0 v1] (576) in
                           5 M-tiles of N=512; straddled tiles evicted
                           with 32-aligned cross-base copies on ScalarE
  q^T,k^T   [96, 4096]     head-dim on partitions (plane-major triplet
                           order so RoPE shifts are 32-row blocks)
  V         [4096, 194]    built by PE transposes of v^T; cols =
                           v_h0|1|v_h1|1 (the 1-columns give the softmax
                           denominator as row 96 of the PV matmul)
  S^T       [tk=128, tq=512] softmax's reduction axis = PE contraction
                           axis -> no transposes in attention.
"""

import sys

sys.path.insert(0, "/opt/trn_rl_repo")

import numpy as np

D_MODEL, N_HEADS, HEAD_DIM, MAX_POS = 1536, 16, 96, 4096
B, T = 2, 2048
NTOK = B * T                      # 4096
NCORES = 8
HPC = N_HEADS // NCORES           # 2 heads per core
NTRIP = HEAD_DIM // 3             # 32 triplets
KT = D_MODEL // 128               # 12 contraction tiles
NCH = NTOK // 512                 # 8 token chunks of 512
TQC = T // 512                    # 4 query chunks per batch
SCALE = 1.0 / np.sqrt(HEAD_DIM)

_CACHE = {}


def _build_nc():
    import concourse.bass as bass
    import concourse.mybir as mybir
    import concourse.tile as tile
    from concourse import bacc

    f32 = mybir.dt.float32
    f32r = mybir.dt.float32r
    MUL = mybir.AluOpType.mult
    ADD = mybir.AluOpType.add

    nc = bacc.Bacc("TRN2", target_bir_lowering=False, debug=False,
                   enable_asserts=False, num_devices=NCORES)

    xT = nc.dram_tensor("xT", [D_MODEL, NTOK], f32r, kind="ExternalInput").ap()
    wallT = nc.dram_tensor("wallT", [D_MODEL, 576], f32r, kind="ExternalInput").ap()
    woT = nc.dram_tensor("woT", [D_MODEL, D_MODEL], f32r, kind="ExternalInput").ap()
    cco = nc.dram_tensor("cco", [96, 3, T], f32, kind="ExternalInput").ap()
    msk = nc.dram_tensor("msk", [128, 4, 512], f32r, kind="ExternalInput").ap()
    idn = nc.dram_tensor("idn", [96, 96], f32r, kind="ExternalInput").ap()
    out = nc.dram_tensor("out", [D_MODEL, 512], f32, kind="ExternalOutput").ap()

    with tile.TileContext(nc) as tc:
        with tc.tile_pool(name="dram", bufs=1, space="DRAM") as dram:
            a2a_in = [dram.tile([NCH, 96, 512], f32r, name=f"a2a_in{h}")
                      for h in range(HPC)]
            a2a_out = [dram.tile([NCH, 96, 512], f32r, name=f"a2a_out{h}")
                       for h in range(HPC)]

            with tc.tile_pool(name="ph12", bufs=1) as pp:
                qk_rot = [pp.tile([96, NTOK], f32r, tag=f"qkrot{i}",
                                  name=f"qkrot{i}") for i in range(4)]
                v_sb = pp.tile([128, NTOK // 128, 194], f32r, tag="vsb")

                # ------------ phase 1: qkv projection + rope ------------
                with tc.tile_pool(name="ph1", bufs=1) as p1, \
                     tc.tile_pool(name="ph1s", bufs=2) as p1s, \
                     tc.tile_pool(name="ps_qk", bufs=6, space="PSUM") as ps_qk, \
                     tc.tile_pool(name="ps_t", bufs=2, space="PSUM") as ps_t:
                    wall_sb = p1.tile([128, KT, 576], f32r, tag="wall")
                    nc.sync.dma_start(
                        wall_sb[:], wallT.rearrange("(k p) c -> p k c", p=128))
                    ident = p1.tile([96, 96], f32r, tag="ident")
                    nc.sync.dma_start(ident[:], idn[:])

                    for ch in range(NCH):
                        coff = (ch % TQC) * 512   # position within batch
                        c_sl = p1s.tile([96, 3, 512], f32, tag="csl")
                        nc.sync.dma_start(c_sl[:], cco[:, :, coff:coff + 512])
                        xt = []
                        for kt in range(KT):
                            t = p1s.tile([128, 512], f32r, tag=f"xt{kt}",
                                         name=f"xt{kt}")
                            nc.sync.dma_start(
                                t[:], xT[kt * 128:(kt + 1) * 128,
                                         ch * 512:(ch + 1) * 512])
                            xt.append(t)
                        # 5 M-tiles over rows [q0 q1 k0 k1 v0 v1] (576)
                        pst = []
                        for m in range(5):
                            mw = 128 if m < 4 else 64
                            ps = ps_qk.tile([128, 512], f32, tag="qk",
                                            name="ps")
                            for kt in range(KT):
                                nc.tensor.matmul(
                                    ps[0:mw, :],
                                    wall_sb[:, kt, m * 128:m * 128 + mw],
                                    xt[kt][:], start=(kt == 0),
                                    stop=(kt == KT - 1))
                            pst.append(ps)
                        # straddled eviction (ScalarE, 32-aligned pieces)
                        raws = []
                        for nm in ("q0", "q1", "k0", "k1"):
                            r = p1s.tile([96, 512], f32, tag=f"raw{nm}",
                                         name=f"raw{nm}")
                            raws.append(r)
                        vr = []
                        for hh in range(HPC):
                            r = p1s.tile([96, 512], f32r, tag=f"vr{hh}",
                                         name=f"vr{hh}")
                            vr.append(r)
                        CP = mybir.ActivationFunctionType.Copy
                        ev = [
                            (raws[0][0:96, :], pst[0][0:96, :]),
                            (raws[1][0:32, :], pst[0][96:128, :]),
                            (raws[1][32:64, :], pst[1][0:32, :]),
                            (raws[1][64:96, :], pst[1][32:64, :]),
                            (raws[2][0:64, :], pst[1][64:128, :]),
                            (raws[2][64:96, :], pst[2][0:32, :]),
                            (raws[3][0:32, :], pst[2][32:64, :]),
                            (raws[3][32:64, :], pst[2][64:96, :]),
                            (raws[3][64:96, :], pst[2][96:128, :]),
                            (vr[0][0:96, :], pst[3][0:96, :]),
                            (vr[1][0:32, :], pst[3][96:128, :]),
                            (vr[1][32:64, :], pst[4][0:32, :]),
                            (vr[1][64:96, :], pst[4][32:64, :]),
                        ]
                        for ei, (dst_ap, src_ap) in enumerate(ev):
                            if ei % 2 == 0:
                                nc.scalar.activation(dst_ap, src_ap, CP)
                            else:
                                nc.vector.tensor_copy(dst_ap, src_ap)
                        # rope on q0,q1,k0,k1
                        for m in range(4):
                            raw = raws[m]
                            dst = qk_rot[m][:, ch * 512:(ch + 1) * 512]
                            nc.vector.tensor_tensor(
                                dst, raw[:], c_sl[:, 0, :], MUL)
                            g1 = p1s.tile([96, 512], f32, tag="g1")
                            nc.sync.dma_start(g1[0:64, :], raw[32:96, :])
                            nc.sync.dma_start(g1[64:96, :], raw[0:32, :])
                            g2 = p1s.tile([96, 512], f32, tag="g2")
                            nc.sync.dma_start(g2[0:32, :], raw[64:96, :])
                            nc.sync.dma_start(g2[32:96, :], raw[0:64, :])
                            nc.vector.tensor_tensor(
                                g1[:], g1[:], c_sl[:, 1, :], MUL)
                            nc.vector.tensor_tensor(dst, dst, g1[:], ADD)
                            nc.vector.tensor_tensor(
                                g2[:], g2[:], c_sl[:, 2, :], MUL)
                            nc.vector.tensor_tensor(dst, dst, g2[:], ADD)
                        # V: PE-transpose v^T -> [tokens, dims]
                        for hh in range(HPC):
                            for ts_ in range(4):
                                pt_ = ps_t.tile([128, 96], f32r, tag="tr",
                                                name="pt_")
                                nc.tensor.transpose(
                                    pt_[:],
                                    vr[hh][:, ts_ * 128:(ts_ + 1) * 128],
                                    ident[:])
                                g = ch * 4 + ts_
                                nc.scalar.activation(
                                    v_sb[:, g, hh * 97:hh * 97 + 96],
                                    pt_[:], CP)
                                nc.vector.memset(
                                    v_sb[:, g, hh * 97 + 96:hh * 97 + 97]
                                    .bitcast(f32), 1.0)

                # ------------ phase 2: attention (+ overlapped A2A/o-proj) --
                with tc.tile_pool(name="ph2", bufs=6) as p2, \
                     tc.tile_pool(name="ph2b", bufs=2) as p2b, \
                     tc.tile_pool(name="ph2c", bufs=1) as p2c, \
                     tc.tile_pool(name="ps_s", bufs=4, space="PSUM") as ps_s, \
                     tc.tile_pool(name="ps_pv", bufs=2, space="PSUM") as ps_pv, \
                     tc.tile_pool(name="ps_o", bufs=2, space="PSUM") as ps_o:
                    m_sb = p2c.tile([128, 4, 512], f32r, tag="msb")
                    nc.sync.dma_start(m_sb[:], msk[:])

                    def attention_chunk(h, b, cl):
                                qoff = b * T + cl * 512
                                pv = ps_pv.tile([128, 512], f32, tag="pv",
                                                name="pv")
                                ntk = 4 * cl + 4
                                order = (list(range(4 * cl, ntk)) +
                                         list(range(4 * cl)))
                                for ti, tt in enumerate(order):
                                    koff = b * T + tt * 128
                                    sp = ps_s.tile([128, 512], f32, tag="s",
                                                   name="sp")
                                    nc.tensor.matmul(
                                        sp[:],
                                        qk_rot[2 + h][:, koff:koff + 128],
                                        qk_rot[h][:, qoff:qoff + 512],
                                        start=True, stop=True)
                                    pt = p2.tile([128, 512], f32r, tag="p",
                                                 name="pt")
                                    nc.scalar.activation(
                                        pt[:], sp[:],
                                        mybir.ActivationFunctionType.Exp)
                                    if tt >= 4 * cl:
                                        nc.vector.tensor_tensor(
                                            pt[:], pt[:],
                                            m_sb[:, tt - 4 * cl, :], MUL)
                                    nc.tensor.matmul(
                                        pv[0:97, :],
                                        v_sb[:, b * 16 + tt,
                                             h * 97:h * 97 + 97],
                                        pt[:], start=(ti == 0),
                                        stop=(ti == ntk - 1))
                                linv = p2b.tile([1, 512], f32, tag="linv",
                                                name="linv")
                                nc.vector.reciprocal(linv[:], pv[96:97, :])
                                brow = p2b.tile([96, 512], f32, tag="brow",
                                                name="brow")
                                nc.gpsimd.partition_broadcast(brow[:], linv[:])
                                att = p2b.tile([96, 512], f32r, tag="att",
                                               name="att")
                                nc.vector.tensor_tensor(
                                    att[:], pv[0:96, :], brow[:], MUL)
                                nc.sync.dma_start(
                                    a2a_in[h][b * TQC + cl, :, :], att[:])

                    def load_att2(half):
                        flat = a2a_out[half][:].rearrange("a b c -> (a b) c")
                        att2 = []
                        for et in range(6):
                            t = p2c.tile([128, 512], f32r,
                                         tag=f"att2_{half}_{et}",
                                         name=f"att2_{half}_{et}")
                            nc.sync.dma_start(
                                t[:], flat[et * 128:(et + 1) * 128, :])
                            att2.append(t)
                        return att2

                    def oproj_group(half, g4, att2, partA_sb):
                        """one group of 2 dt tiles x full 6-K-tile half."""
                        pos = [ps_o.tile([128, 512], f32, tag="o",
                                         name=f"po_{half}_{g4}_{i}")
                               for i in range(2)]
                        for et in range(6):
                            wot = p2c.tile([128, 256], f32r, tag="wos",
                                           bufs=4, name="wot")
                            nc.sync.dma_start(
                                wot[:],
                                woT[half * 768 + et * 128:
                                    half * 768 + (et + 1) * 128,
                                    g4 * 256:(g4 + 1) * 256])
                            for i in range(2):
                                nc.tensor.matmul(
                                    pos[i][:],
                                    wot[:, i * 128:(i + 1) * 128],
                                    att2[et][:], start=(et == 0),
                                    stop=(et == 5),
                                    skip_group_check=True)
                        for i in range(2):
                            dt_ = g4 * 2 + i
                            if half == 0:
                                nc.vector.tensor_copy(
                                    partA_sb[:, dt_, :], pos[i][:])
                            else:
                                ot = p2b.tile([128, 512], f32, tag="ot",
                                              bufs=3, name="ot")
                                nc.vector.tensor_tensor(
                                    ot[:], pos[i][:],
                                    partA_sb[:, dt_, :], ADD)
                                nc.sync.dma_start(
                                    out[dt_ * 128:(dt_ + 1) * 128, :],
                                    ot[:])

                    partA_sb = p2c.tile([128, KT, 512], f32, tag="partA")
                    for b in range(B):
                        for cl in range(TQC):
                            attention_chunk(0, b, cl)
                    nc.gpsimd.collective_compute(
                        "AllToAll", mybir.AluOpType.bypass,
                        replica_groups=[list(range(NCORES))],
                        ins=[a2a_in[0].opt()], outs=[a2a_out[0].opt()])
                    att2A = load_att2(0)
                    chunks = [(b, cl) for b in range(B) for cl in range(TQC)]
                    for idx, (b, cl) in enumerate(chunks):
                        attention_chunk(1, b, cl)
                        g4 = idx - 2
                        if 0 <= g4 < 6:
                            oproj_group(0, g4, att2A, partA_sb)
                    nc.gpsimd.collective_compute(
                        "AllToAll", mybir.AluOpType.bypass,
                        replica_groups=[list(range(NCORES))],
                        ins=[a2a_in[1].opt()], outs=[a2a_out[1].opt()])
                    att2B = load_att2(1)
                    for g4 in range(6):
                        oproj_group(1, g4, att2B, partA_sb)

    nc.compile()
    return nc


def _plane_major(w):
    """Reorder head-dim rows 3k+i -> 32i+k (per 96-row head block)."""
    idx = np.empty(96, dtype=np.int64)
    for i in range(3):
        for k in range(NTRIP):
            idx[32 * i + k] = 3 * k + i
    return w[idx]


def _prep_inputs(x, w_qkv, w_o, Rs):
    x = np.asarray(x, dtype=np.float32)
    w_qkv = np.asarray(w_qkv, dtype=np.float32)
    w_o = np.asarray(w_o, dtype=np.float32)
    Rs = np.asarray(Rs, dtype=np.float32)

    xT = np.ascontiguousarray(x.reshape(NTOK, D_MODEL).T)

    # rope coefficients, plane-major rows: C[d, delta, t]
    R = Rs[:T]                                   # (T, 32, 3, 3)
    cco = np.empty((96, 3, T), dtype=np.float32)
    for d in range(3):
        for i in range(3):
            cco[32 * i:32 * i + 32, d, :] = R[:, :, i, (i + d) % 3].T

    # causal masks for the 4 diagonal sub-tiles
    msk = np.empty((128, 4, 512), dtype=np.float32)
    j = np.arange(128)[:, None]
    i = np.arange(512)[None, :]
    for m in range(4):
        msk[:, m, :] = (m * 128 + j <= i).astype(np.float32)

    # w_o columns-for-even-heads first, then odd (matches split A2A halves)
    woT = np.ascontiguousarray(w_o.T)            # rows e = h*96+d
    perm = np.concatenate(
        [np.arange(h * 96, (h + 1) * 96) for h in range(0, 16, 2)] +
        [np.arange(h * 96, (h + 1) * 96) for h in range(1, 16, 2)])
    woTp = np.ascontiguousarray(woT[perm])

    def w_row(s, h):                             # rows of w_qkv for (q/k/v, head)
        base = (s * N_HEADS + h) * HEAD_DIM
        return w_qkv[base:base + HEAD_DIM]

    in_maps = []
    for c in range(NCORES):
        h0, h1 = 2 * c, 2 * c + 1
        wall = np.concatenate([
            _plane_major(w_row(0, h0)) * SCALE,
            _plane_major(w_row(0, h1)) * SCALE,
            _plane_major(w_row(1, h0)),
            _plane_major(w_row(1, h1)),
            w_row(2, h0),
            w_row(2, h1),
        ], axis=0)                               # [576, 1536]
        wallT = np.ascontiguousarray(wall.T)     # [1536, 576]
        in_maps.append({
            "xT": xT, "wallT": wallT, "woT": woTp,
            "cco": cco, "msk": msk, "idn": np.eye(96, dtype=np.float32),
        })
    return in_maps


def kernel(x, w_qkv, w_o, Rs):
    from concourse import bass_utils

    if "nc" not in _CACHE:
        _CACHE["nc"] = _build_nc()
    nc = _CACHE["nc"]
    in_maps = _prep_inputs(x, w_qkv, w_o, Rs)
    res = bass_utils.run_bass_kernel_spmd(
        nc, in_maps, core_ids=list(range(NCORES)))
    full_T = np.concatenate([res.results[c]["out"] for c in range(NCORES)],
                            axis=1)              # [1536, 4096]
    return np.ascontiguousarray(full_T.T).reshape(B, T, D_MODEL)


# revision 12
# speedup vs baseline: 1.1436x; 1.0151x over previous
"""Trainium2 Bass kernel: causal attention with 3D (Rodrigues) RoPE.

Sharding: tensor-parallel over heads (2 heads/core on 8 cores) for
QKV projection + RoPE + SDPA, then an AllToAll redistributes attention
outputs so the output projection is sharded over tokens (512/core).
The A2A is split per local head so the first half overlaps attention,
and the output projection's contraction is split to match.

Layouts (per core, all matmuls in float32r):
  x^T       [1536, 4096]   tokens on the free axis
  qkv proj  computed transposed: out rows [q0 q1 k0 k1 v0 v1] (576) in
                           5 M-tiles of N=512; straddled tiles evicted
                           with 32-aligned cross-base copies on ScalarE
  q^T,k^T   [96, 4096]     head-dim on partitions (plane-major triplet
                           order so RoPE shifts are 32-row blocks)
  V         [4096, 194]    built by PE transposes of v^T; cols =
                           v_h0|1|v_h1|1 (the 1-columns give the softmax
                           denominator as row 96 of the PV matmul)
  S^T       [tk=128, tq=512] softmax's reduction axis = PE contraction
                           axis -> no transposes in attention.
"""

import sys

sys.path.insert(0, "/opt/trn_rl_repo")

import numpy as np

D_MODEL, N_HEADS, HEAD_DIM, MAX_POS = 1536, 16, 96, 4096
B, T = 2, 2048
NTOK = B * T                      # 4096
NCORES = 8
HPC = N_HEADS // NCORES           # 2 heads per core
NTRIP = HEAD_DIM // 3             # 32 triplets
KT = D_MODEL // 128               # 12 contraction tiles
NCH = NTOK // 512                 # 8 token chunks of 512
TQC = T // 512                    # 4 query chunks per batch
SCALE = 1.0 / np.sqrt(HEAD_DIM)

_CACHE = {}


def _build_nc():
    import concourse.bass as bass
    import concourse.mybir as mybir
    import concourse.tile as tile
    from concourse import bacc

    f32 = mybir.dt.float32
    f32r = mybir.dt.float32r
    MUL = mybir.AluOpType.mult
    ADD = mybir.AluOpType.add

    nc = bacc.Bacc("TRN2", target_bir_lowering=False, debug=False,
                   enable_asserts=False, num_devices=NCORES)

    xT = nc.dram_tensor("xT", [D_MODEL, NTOK], f32r, kind="ExternalInput").ap()
    wallT = nc.dram_tensor("wallT", [D_MODEL, 576], f32r, kind="ExternalInput").ap()
    woT = nc.dram_tensor("woT", [D_MODEL, D_MODEL], f32r, kind="ExternalInput").ap()
    cco = nc.dram_tensor("cco", [96, 3, T], f32, kind="ExternalInput").ap()
    msk = nc.dram_tensor("msk", [128, 4, 512], f32r, kind="ExternalInput").ap()
    idn = nc.dram_tensor("idn", [96, 96], f32r, kind="ExternalInput").ap()
    out = nc.dram_tensor("out", [D_MODEL, 512], f32, kind="ExternalOutput").ap()

    with tile.TileContext(nc) as tc:
        with tc.tile_pool(name="dram", bufs=1, space="DRAM") as dram:
            a2a_in = [dram.tile([NCH, 96, 512], f32r, name=f"a2a_in{h}")
                      for h in range(HPC)]
            a2a_out = [dram.tile([NCH, 96, 512], f32r, name=f"a2a_out{h}")
                       for h in range(HPC)]

            with tc.tile_pool(name="ph12", bufs=1) as pp:
                qk_rot = [pp.tile([96, NTOK], f32r, tag=f"qkrot{i}",
                                  name=f"qkrot{i}") for i in range(4)]
                v_sb = pp.tile([128, NTOK // 128, 194], f32r, tag="vsb")

                # ------------ phase 1: qkv projection + rope ------------
                with tc.tile_pool(name="ph1", bufs=1) as p1, \
                     tc.tile_pool(name="ph1s", bufs=2) as p1s, \
                     tc.tile_pool(name="ps_qk", bufs=6, space="PSUM") as ps_qk, \
                     tc.tile_pool(name="ps_t", bufs=2, space="PSUM") as ps_t:
                    wall_sb = p1.tile([128, KT, 576], f32r, tag="wall")
                    nc.sync.dma_start(
                        wall_sb[:], wallT.rearrange("(k p) c -> p k c", p=128))
                    ident = p1.tile([96, 96], f32r, tag="ident")
                    nc.sync.dma_start(ident[:], idn[:])

                    prev_vr = [None]
                    CP = mybir.ActivationFunctionType.Copy

                    def emit_transposes(chp, vrp):
                        for hh in range(HPC):
                            for ts_ in range(4):
                                pt_ = ps_t.tile([128, 96], f32r, tag="tr",
                                                name="pt_")
                                nc.tensor.transpose(
                                    pt_[:],
                                    vrp[hh][:, ts_ * 128:(ts_ + 1) * 128],
                                    ident[:])
                                g = chp * 4 + ts_
                                nc.scalar.activation(
                                    v_sb[:, g, hh * 97:hh * 97 + 96],
                                    pt_[:], CP)
                                nc.vector.memset(
                                    v_sb[:, g, hh * 97 + 96:hh * 97 + 97]
                                    .bitcast(f32), 1.0)

                    for ch in range(NCH):
                        coff = (ch % TQC) * 512   # position within batch
                        c_sl = p1s.tile([96, 3, 512], f32, tag="csl")
                        nc.sync.dma_start(c_sl[:], cco[:, :, coff:coff + 512])
                        xt = []
                        for kt in range(KT):
                            t = p1s.tile([128, 512], f32r, tag=f"xt{kt}",
                                         name=f"xt{kt}")
                            nc.sync.dma_start(
                                t[:], xT[kt * 128:(kt + 1) * 128,
                                         ch * 512:(ch + 1) * 512])
                            xt.append(t)
                        # 5 M-tiles over rows [q0 q1 k0 k1 v0 v1] (576)
                        pst = []
                        for m in range(5):
                            mw = 128 if m < 4 else 64
                            ps = ps_qk.tile([128, 512], f32, tag="qk",
                                            name="ps")
                            for kt in range(KT):
                                nc.tensor.matmul(
                                    ps[0:mw, :],
                                    wall_sb[:, kt, m * 128:m * 128 + mw],
                                    xt[kt][:], start=(kt == 0),
                                    stop=(kt == KT - 1))
                            pst.append(ps)
                        # straddled eviction (ScalarE, 32-aligned pieces)
                        raws = []
                        for nm in ("q0", "q1", "k0", "k1"):
                            r = p1s.tile([96, 512], f32, tag=f"raw{nm}",
                                         name=f"raw{nm}")
                            raws.append(r)
                        vr = []
                        for hh in range(HPC):
                            r = p1s.tile([96, 512], f32r, tag=f"vr{hh}",
                                         name=f"vr{hh}")
                            vr.append(r)
                        ev = [
                            (raws[0][0:96, :], pst[0][0:96, :]),
                            (raws[1][0:32, :], pst[0][96:128, :]),
                            (raws[1][32:64, :], pst[1][0:32, :]),
                            (raws[1][64:96, :], pst[1][32:64, :]),
                            (raws[2][0:64, :], pst[1][64:128, :]),
                            (raws[2][64:96, :], pst[2][0:32, :]),
                            (raws[3][0:32, :], pst[2][32:64, :]),
                            (raws[3][32:64, :], pst[2][64:96, :]),
                            (raws[3][64:96, :], pst[2][96:128, :]),
                            (vr[0][0:96, :], pst[3][0:96, :]),
                            (vr[1][0:32, :], pst[3][96:128, :]),
                            (vr[1][32:64, :], pst[4][0:32, :]),
                            (vr[1][64:96, :], pst[4][32:64, :]),
                        ]
                        for ei, (dst_ap, src_ap) in enumerate(ev):
                            if ei % 2 == 0:
                                nc.scalar.activation(dst_ap, src_ap, CP)
                            else:
                                nc.vector.tensor_copy(dst_ap, src_ap)
                        # rope on q0,q1,k0,k1
                        for m in range(4):
                            raw = raws[m]
                            dst = qk_rot[m][:, ch * 512:(ch + 1) * 512]
                            nc.vector.tensor_tensor(
                                dst, raw[:], c_sl[:, 0, :], MUL)
                            g1 = p1s.tile([96, 512], f32, tag="g1")
                            nc.sync.dma_start(g1[0:64, :], raw[32:96, :])
                            nc.sync.dma_start(g1[64:96, :], raw[0:32, :])
                            g2 = p1s.tile([96, 512], f32, tag="g2")
                            nc.sync.dma_start(g2[0:32, :], raw[64:96, :])
                            nc.sync.dma_start(g2[32:96, :], raw[0:64, :])
                            nc.vector.tensor_tensor(
                                g1[:], g1[:], c_sl[:, 1, :], MUL)
                            nc.vector.tensor_tensor(dst, dst, g1[:], ADD)
                            nc.vector.tensor_tensor(
                                g2[:], g2[:], c_sl[:, 2, :], MUL)
                            nc.vector.tensor_tensor(dst, dst, g2[:], ADD)
                        # V: PE-transpose v^T of the PREVIOUS chunk so the
                        # transposes interleave with this chunk's projections
                        if ch > 0:
                            emit_transposes(ch - 1, prev_vr[0])
                        prev_vr[0] = vr
                    emit_transposes(NCH - 1, prev_vr[0])

                # ------------ phase 2: attention (+ overlapped A2A/o-proj) --
                with tc.tile_pool(name="ph2", bufs=6) as p2, \
                     tc.tile_pool(name="ph2b", bufs=2) as p2b, \
                     tc.tile_pool(name="ph2c", bufs=1) as p2c, \
                     tc.tile_pool(name="ps_s", bufs=4, space="PSUM") as ps_s, \
                     tc.tile_pool(name="ps_pv", bufs=2, space="PSUM") as ps_pv, \
                     tc.tile_pool(name="ps_o", bufs=2, space="PSUM") as ps_o:
                    m_sb = p2c.tile([128, 4, 512], f32r, tag="msb")
                    nc.sync.dma_start(m_sb[:], msk[:])

                    def attention_chunk(h, b, cl):
                                qoff = b * T + cl * 512
                                pv = ps_pv.tile([128, 512], f32, tag="pv",
                                                name="pv")
                                ntk = 4 * cl + 4
                                order = (list(range(4 * cl, ntk)) +
                                         list(range(4 * cl)))
                                for ti, tt in enumerate(order):
                                    koff = b * T + tt * 128
                                    sp = ps_s.tile([128, 512], f32, tag="s",
                                                   name="sp")
                                    nc.tensor.matmul(
                                        sp[:],
                                        qk_rot[2 + h][:, koff:koff + 128],
                                        qk_rot[h][:, qoff:qoff + 512],
                                        start=True, stop=True)
                                    pt = p2.tile([128, 512], f32r, tag="p",
                                                 name="pt")
                                    nc.scalar.activation(
                                        pt[:], sp[:],
                                        mybir.ActivationFunctionType.Exp)
                                    if tt >= 4 * cl:
                                        nc.vector.tensor_tensor(
                                            pt[:], pt[:],
                                            m_sb[:, tt - 4 * cl, :], MUL)
                                    nc.tensor.matmul(
                                        pv[0:97, :],
                                        v_sb[:, b * 16 + tt,
                                             h * 97:h * 97 + 97],
                                        pt[:], start=(ti == 0),
                                        stop=(ti == ntk - 1))
                                linv = p2b.tile([1, 512], f32, tag="linv",
                                                name="linv")
                                nc.vector.reciprocal(linv[:], pv[96:97, :])
                                brow = p2b.tile([96, 512], f32, tag="brow",
                                                name="brow")
                                nc.gpsimd.partition_broadcast(brow[:], linv[:])
                                att = p2b.tile([96, 512], f32r, tag="att",
                                               name="att")
                                nc.vector.tensor_tensor(
                                    att[:], pv[0:96, :], brow[:], MUL)
                                nc.sync.dma_start(
                                    a2a_in[h][b * TQC + cl, :, :], att[:])

                    def load_att2(half):
                        flat = a2a_out[half][:].rearrange("a b c -> (a b) c")
                        att2 = []
                        for et in range(6):
                            t = p2c.tile([128, 512], f32r,
                                         tag=f"att2_{half}_{et}",
                                         name=f"att2_{half}_{et}")
                            nc.sync.dma_start(
                                t[:], flat[et * 128:(et + 1) * 128, :])
                            att2.append(t)
                        return att2

                    def oproj_group(half, g4, att2, partA_sb):
                        """one group of 2 dt tiles x full 6-K-tile half."""
                        pos = [ps_o.tile([128, 512], f32, tag="o",
                                         name=f"po_{half}_{g4}_{i}")
                               for i in range(2)]
                        for et in range(6):
                            wot = p2c.tile([128, 256], f32r, tag="wos",
                                           bufs=4, name="wot")
                            nc.sync.dma_start(
                                wot[:],
                                woT[half * 768 + et * 128:
                                    half * 768 + (et + 1) * 128,
                                    g4 * 256:(g4 + 1) * 256])
                            for i in range(2):
                                nc.tensor.matmul(
                                    pos[i][:],
                                    wot[:, i * 128:(i + 1) * 128],
                                    att2[et][:], start=(et == 0),
                                    stop=(et == 5),
                                    skip_group_check=True)
                        for i in range(2):
                            dt_ = g4 * 2 + i
                            if half == 0:
                                nc.vector.tensor_copy(
                                    partA_sb[:, dt_, :], pos[i][:])
                            else:
                                ot = p2b.tile([128, 512], f32, tag="ot",
                                              bufs=3, name="ot")
                                nc.vector.tensor_tensor(
                                    ot[:], pos[i][:],
                                    partA_sb[:, dt_, :], ADD)
                                nc.sync.dma_start(
                                    out[dt_ * 128:(dt_ + 1) * 128, :],
                                    ot[:])

                    partA_sb = p2c.tile([128, KT, 512], f32, tag="partA")
                    for b in range(B):
                        for cl in range(TQC):
                            attention_chunk(0, b, cl)
                    nc.gpsimd.collective_compute(
                        "AllToAll", mybir.AluOpType.bypass,
                        replica_groups=[list(range(NCORES))],
                        ins=[a2a_in[0].opt()], outs=[a2a_out[0].opt()])
                    att2A = load_att2(0)
                    for b in range(B):
                        for cl in range(TQC):
                            attention_chunk(1, b, cl)
                    # A2A#2 runs on the collective engine while PE does oproj0
                    nc.gpsimd.collective_compute(
                        "AllToAll", mybir.AluOpType.bypass,
                        replica_groups=[list(range(NCORES))],
                        ins=[a2a_in[1].opt()], outs=[a2a_out[1].opt()])
                    for g4 in range(6):
                        oproj_group(0, g4, att2A, partA_sb)
                    att2B = load_att2(1)
                    for g4 in range(6):
                        oproj_group(1, g4, att2B, partA_sb)

    nc.compile()
    return nc


def _plane_major(w):
    """Reorder head-dim rows 3k+i -> 32i+k (per 96-row head block)."""
    idx = np.empty(96, dtype=np.int64)
    for i in range(3):
        for k in range(NTRIP):
            idx[32 * i + k] = 3 * k + i
    return w[idx]


def _prep_inputs(x, w_qkv, w_o, Rs):
    x = np.asarray(x, dtype=np.float32)
    w_qkv = np.asarray(w_qkv, dtype=np.float32)
    w_o = np.asarray(w_o, dtype=np.float32)
    Rs = np.asarray(Rs, dtype=np.float32)

    xT = np.ascontiguousarray(x.reshape(NTOK, D_MODEL).T)

    # rope coefficients, plane-major rows: C[d, delta, t]
    R = Rs[:T]                                   # (T, 32, 3, 3)
    cco = np.empty((96, 3, T), dtype=np.float32)
    for d in range(3):
        for i in range(3):
            cco[32 * i:32 * i + 32, d, :] = R[:, :, i, (i + d) % 3].T

    # causal masks for the 4 diagonal sub-tiles
    msk = np.empty((128, 4, 512), dtype=np.float32)
    j = np.arange(128)[:, None]
    i = np.arange(512)[None, :]
    for m in range(4):
        msk[:, m, :] = (m * 128 + j <= i).astype(np.float32)

    # w_o columns-for-even-heads first, then odd (matches split A2A halves)
    woT = np.ascontiguousarray(w_o.T)            # rows e = h*96+d
    perm = np.concatenate(
        [np.arange(h * 96, (h + 1) * 96) for h in range(0, 16, 2)] +
        [np.arange(h * 96, (h + 1) * 96) for h in range(1, 16, 2)])
    woTp = np.ascontiguousarray(woT[perm])

    def w_row(s, h):                             # rows of w_qkv for (q/k/v, head)
        base = (s * N_HEADS + h) * HEAD_DIM
        return w_qkv[base:base + HEAD_DIM]

    in_maps = []
    for c in range(NCORES):
        h0, h1 = 2 * c, 2 * c + 1
        wall = np.concatenate([
            _plane_major(w_row(0, h0)) * SCALE,
            _plane_major(w_row(0, h1)) * SCALE,
            _plane_major(w_row(1, h0)),
            _plane_major(w_row(1, h1)),
            w_row(2, h0),
            w_row(2, h1),
        ], axis=0)                               # [576, 1536]
        wallT = np.ascontiguousarray(wall.T)     # [1536, 576]
        in_maps.append({
            "xT": xT, "wallT": wallT, "woT": woTp,
            "cco": cco, "msk": msk, "idn": np.eye(96, dtype=np.float32),
        })
    return in_maps


def kernel(x, w_qkv, w_o, Rs):
    from concourse import bass_utils

    if "nc" not in _CACHE:
        _CACHE["nc"] = _build_nc()
    nc = _CACHE["nc"]
    in_maps = _prep_inputs(x, w_qkv, w_o, Rs)
    res = bass_utils.run_bass_kernel_spmd(
        nc, in_maps, core_ids=list(range(NCORES)))
    full_T = np.concatenate([res.results[c]["out"] for c in range(NCORES)],
                            axis=1)              # [1536, 4096]
    return np.ascontiguousarray(full_T.T).reshape(B, T, D_MODEL)


# revision 13
# speedup vs baseline: 1.3114x; 1.1467x over previous
"""Trainium2 Bass kernel: causal attention with 3D (Rodrigues) RoPE.

Sharding: tensor-parallel over heads (2 heads/core on 8 cores) for
QKV projection + RoPE + SDPA, then an AllToAll redistributes attention
outputs so the output projection is sharded over tokens (512/core).
The A2A is split per local head so the first half overlaps attention,
and the output projection's contraction is split to match.

Layouts (per core, all matmuls in float32r):
  x^T       [1536, 4096]   tokens on the free axis
  qkv proj  computed transposed: out rows [q0 q1 k0 k1 v0 v1] (576) in
                           5 M-tiles of N=512; straddled tiles evicted
                           with 32-aligned cross-base copies on ScalarE
  q^T,k^T   [96, 4096]     head-dim on partitions (plane-major triplet
                           order so RoPE shifts are 32-row blocks)
  V         [4096, 194]    built by PE transposes of v^T; cols =
                           v_h0|1|v_h1|1 (the 1-columns give the softmax
                           denominator as row 96 of the PV matmul)
  S^T       [tk=128, tq=512] softmax's reduction axis = PE contraction
                           axis -> no transposes in attention.
"""

import sys

sys.path.insert(0, "/opt/trn_rl_repo")

import numpy as np

D_MODEL, N_HEADS, HEAD_DIM, MAX_POS = 1536, 16, 96, 4096
B, T = 2, 2048
NTOK = B * T                      # 4096
NCORES = 8
HPC = N_HEADS // NCORES           # 2 heads per core
NTRIP = HEAD_DIM // 3             # 32 triplets
KT = D_MODEL // 128               # 12 contraction tiles
NCH = NTOK // 512                 # 8 token chunks of 512
TQC = T // 512                    # 4 query chunks per batch
SCALE = 1.0 / np.sqrt(HEAD_DIM)

_CACHE = {}


def _build_nc():
    import concourse.bass as bass
    import concourse.mybir as mybir
    import concourse.tile as tile
    from concourse import bacc

    f32 = mybir.dt.float32
    f32r = mybir.dt.float32r
    MUL = mybir.AluOpType.mult
    ADD = mybir.AluOpType.add

    nc = bacc.Bacc("TRN2", target_bir_lowering=False, debug=False,
                   enable_asserts=False, num_devices=NCORES)

    xT = nc.dram_tensor("xT", [D_MODEL, NTOK], f32r, kind="ExternalInput").ap()
    wallT = nc.dram_tensor("wallT", [D_MODEL, 576], f32r, kind="ExternalInput").ap()
    woT = nc.dram_tensor("woT", [D_MODEL, D_MODEL], f32r, kind="ExternalInput").ap()
    cco = nc.dram_tensor("cco", [96, 3, T], f32, kind="ExternalInput").ap()
    msk = nc.dram_tensor("msk", [128, 4, 512], f32r, kind="ExternalInput").ap()
    idn = nc.dram_tensor("idn", [96, 96], f32r, kind="ExternalInput").ap()
    out = nc.dram_tensor("out", [D_MODEL, 512], f32, kind="ExternalOutput").ap()

    with tile.TileContext(nc) as tc:
        with tc.tile_pool(name="dram", bufs=1, space="DRAM") as dram:
            a2a_in = [dram.tile([NCH, 96, 512], f32r, name=f"a2a_in{h}")
                      for h in range(HPC)]
            a2a_out = [dram.tile([NCH, 96, 512], f32r, name=f"a2a_out{h}")
                       for h in range(HPC)]

            with tc.tile_pool(name="ph12", bufs=1) as pp:
                qk_rot = [pp.tile([96, NTOK], f32r, tag=f"qkrot{i}",
                                  name=f"qkrot{i}") for i in range(4)]
                v_sb = pp.tile([128, NTOK // 128, 194], f32r, tag="vsb")

                # ------------ phase 1: qkv projection + rope ------------
                with tc.tile_pool(name="ph1", bufs=1) as p1, \
                     tc.tile_pool(name="ph1s", bufs=2) as p1s, \
                     tc.tile_pool(name="ps_qk", bufs=6, space="PSUM") as ps_qk, \
                     tc.tile_pool(name="ps_t", bufs=2, space="PSUM") as ps_t:
                    wall_sb = p1.tile([128, KT, 576], f32r, tag="wall")
                    nc.sync.dma_start(
                        wall_sb[:], wallT.rearrange("(k p) c -> p k c", p=128))
                    ident = p1.tile([96, 96], f32r, tag="ident")
                    nc.sync.dma_start(ident[:], idn[:])

                    prev_vr = [None]
                    CP = mybir.ActivationFunctionType.Copy

                    def emit_transposes(chp, vrp):
                        for hh in range(HPC):
                            for ts_ in range(4):
                                pt_ = ps_t.tile([128, 96], f32r, tag="tr",
                                                name="pt_")
                                nc.tensor.transpose(
                                    pt_[:],
                                    vrp[hh][:, ts_ * 128:(ts_ + 1) * 128],
                                    ident[:])
                                g = chp * 4 + ts_
                                nc.scalar.activation(
                                    v_sb[:, g, hh * 97:hh * 97 + 96],
                                    pt_[:], CP)
                                nc.vector.memset(
                                    v_sb[:, g, hh * 97 + 96:hh * 97 + 97]
                                    .bitcast(f32), 1.0)

                    for ch in range(NCH):
                        coff = (ch % TQC) * 512   # position within batch
                        c_sl = p1s.tile([96, 3, 512], f32, tag="csl")
                        nc.sync.dma_start(c_sl[:], cco[:, :, coff:coff + 512])
                        xt = []
                        for kt in range(KT):
                            t = p1s.tile([128, 512], f32r, tag=f"xt{kt}",
                                         name=f"xt{kt}")
                            nc.sync.dma_start(
                                t[:], xT[kt * 128:(kt + 1) * 128,
                                         ch * 512:(ch + 1) * 512])
                            xt.append(t)
                        # 5 M-tiles over rows [q0 q1 k0 k1 v0 v1] (576)
                        pst = []
                        for m in range(5):
                            mw = 128 if m < 4 else 64
                            ps = ps_qk.tile([128, 512], f32, tag="qk",
                                            name="ps")
                            for kt in range(KT):
                                nc.tensor.matmul(
                                    ps[0:mw, :],
                                    wall_sb[:, kt, m * 128:m * 128 + mw],
                                    xt[kt][:], start=(kt == 0),
                                    stop=(kt == KT - 1))
                            pst.append(ps)
                        # straddled eviction (ScalarE, 32-aligned pieces)
                        raws = []
                        for nm in ("q0", "q1", "k0", "k1"):
                            r = p1s.tile([96, 512], f32, tag=f"raw{nm}",
                                         name=f"raw{nm}")
                            raws.append(r)
                        vr = []
                        for hh in range(HPC):
                            r = p1s.tile([96, 512], f32r, tag=f"vr{hh}",
                                         name=f"vr{hh}")
                            vr.append(r)
                        ev = [
                            (raws[0][0:96, :], pst[0][0:96, :]),
                            (raws[1][0:32, :], pst[0][96:128, :]),
                            (raws[1][32:64, :], pst[1][0:32, :]),
                            (raws[1][64:96, :], pst[1][32:64, :]),
                            (raws[2][0:64, :], pst[1][64:128, :]),
                            (raws[2][64:96, :], pst[2][0:32, :]),
                            (raws[3][0:32, :], pst[2][32:64, :]),
                            (raws[3][32:64, :], pst[2][64:96, :]),
                            (raws[3][64:96, :], pst[2][96:128, :]),
                            (vr[0][0:96, :], pst[3][0:96, :]),
                            (vr[1][0:32, :], pst[3][96:128, :]),
                            (vr[1][32:64, :], pst[4][0:32, :]),
                            (vr[1][64:96, :], pst[4][32:64, :]),
                        ]
                        for ei, (dst_ap, src_ap) in enumerate(ev):
                            if ei % 2 == 0:
                                nc.scalar.activation(dst_ap, src_ap, CP)
                            else:
                                nc.vector.tensor_copy(dst_ap, src_ap)
                        # rope on q0,q1,k0,k1
                        for m in range(4):
                            raw = raws[m]
                            dst = qk_rot[m][:, ch * 512:(ch + 1) * 512]
                            nc.vector.tensor_tensor(
                                dst, raw[:], c_sl[:, 0, :], MUL)
                            g1 = p1s.tile([96, 512], f32, tag="g1")
                            nc.gpsimd.dma_start(g1[0:64, :], raw[32:96, :])
                            nc.gpsimd.dma_start(g1[64:96, :], raw[0:32, :])
                            g2 = p1s.tile([96, 512], f32, tag="g2")
                            nc.gpsimd.dma_start(g2[0:32, :], raw[64:96, :])
                            nc.gpsimd.dma_start(g2[32:96, :], raw[0:64, :])
                            nc.vector.tensor_tensor(
                                g1[:], g1[:], c_sl[:, 1, :], MUL)
                            nc.vector.tensor_tensor(dst, dst, g1[:], ADD)
                            nc.vector.tensor_tensor(
                                g2[:], g2[:], c_sl[:, 2, :], MUL)
                            nc.vector.tensor_tensor(dst, dst, g2[:], ADD)
                        # V: PE-transpose v^T of the PREVIOUS chunk so the
                        # transposes interleave with this chunk's projections
                        if ch > 0:
                            emit_transposes(ch - 1, prev_vr[0])
                        prev_vr[0] = vr
                    emit_transposes(NCH - 1, prev_vr[0])

                # ------------ phase 2: attention (+ overlapped A2A/o-proj) --
                with tc.tile_pool(name="ph2", bufs=6) as p2, \
                     tc.tile_pool(name="ph2b", bufs=3) as p2b, \
                     tc.tile_pool(name="ph2c", bufs=1) as p2c:
                    m_sb = p2c.tile([128, 4, 512], f32r, tag="msb")
                    nc.sync.dma_start(m_sb[:], msk[:])

                    def attention_chunk(h, b, cl, ps_s, ps_pv):
                                qoff = b * T + cl * 512
                                pv = ps_pv.tile([128, 512], f32, tag="pv",
                                                name="pv")
                                ntk = 4 * cl + 4
                                order = (list(range(4 * cl, ntk)) +
                                         list(range(4 * cl)))
                                pend = []      # (ti, tt, pt) awaiting PV

                                def emit_pv():
                                    ti, tt, pt = pend.pop(0)
                                    nc.tensor.matmul(
                                        pv[0:97, :],
                                        v_sb[:, b * 16 + tt,
                                             h * 97:h * 97 + 97],
                                        pt[:], start=(ti == 0),
                                        stop=(ti == ntk - 1))

                                for ti, tt in enumerate(order):
                                    koff = b * T + tt * 128
                                    sp = ps_s.tile([128, 512], f32, tag="s",
                                                   name="sp")
                                    nc.tensor.matmul(
                                        sp[:],
                                        qk_rot[2 + h][:, koff:koff + 128],
                                        qk_rot[h][:, qoff:qoff + 512],
                                        start=True, stop=True)
                                    pt = p2.tile([128, 512], f32r, tag="p",
                                                 name="pt")
                                    nc.scalar.activation(
                                        pt[:], sp[:],
                                        mybir.ActivationFunctionType.Exp)
                                    if tt >= 4 * cl:
                                        nc.vector.tensor_tensor(
                                            pt[:], pt[:],
                                            m_sb[:, tt - 4 * cl, :], MUL)
                                    pend.append((ti, tt, pt))
                                    if len(pend) > 2:
                                        emit_pv()
                                while pend:
                                    emit_pv()
                                linv = p2b.tile([1, 512], f32, tag="linv",
                                                name="linv")
                                nc.vector.reciprocal(linv[:], pv[96:97, :])
                                brow = p2b.tile([96, 512], f32, tag="brow",
                                                name="brow")
                                nc.gpsimd.partition_broadcast(brow[:], linv[:])
                                att = p2b.tile([96, 512], f32r, tag="att",
                                               name="att")
                                nc.vector.tensor_tensor(
                                    att[:], pv[0:96, :], brow[:], MUL)
                                nc.sync.dma_start(
                                    a2a_in[h][b * TQC + cl, :, :], att[:])

                    def load_att2(half):
                        flat = a2a_out[half][:].rearrange("a b c -> (a b) c")
                        att2 = []
                        for et in range(6):
                            t = p2c.tile([128, 512], f32r,
                                         tag=f"att2_{half}_{et}",
                                         name=f"att2_{half}_{et}")
                            nc.sync.dma_start(
                                t[:], flat[et * 128:(et + 1) * 128, :])
                            att2.append(t)
                        return att2

                    def oproj_group(half, g4, att2, partA_sb, ps_o):
                        """one group of 4 dt tiles x full 6-K-tile half."""
                        pos = [ps_o.tile([128, 512], f32, tag="o",
                                         name=f"po_{half}_{g4}_{i}")
                               for i in range(4)]
                        for et in range(6):
                            wot = p2c.tile([128, 512], f32r, tag="wos",
                                           bufs=4, name="wot")
                            nc.sync.dma_start(
                                wot[:],
                                woT[half * 768 + et * 128:
                                    half * 768 + (et + 1) * 128,
                                    g4 * 512:(g4 + 1) * 512])
                            for i in range(4):
                                nc.tensor.matmul(
                                    pos[i][:],
                                    wot[:, i * 128:(i + 1) * 128],
                                    att2[et][:], start=(et == 0),
                                    stop=(et == 5),
                                    skip_group_check=True)
                        for i in range(4):
                            dt_ = g4 * 4 + i
                            if half == 0:
                                nc.vector.tensor_copy(
                                    partA_sb[:, dt_, :], pos[i][:])
                            else:
                                ot = p2b.tile([128, 512], f32, tag="ot",
                                              bufs=3, name="ot")
                                nc.vector.tensor_tensor(
                                    ot[:], pos[i][:],
                                    partA_sb[:, dt_, :], ADD)
                                nc.sync.dma_start(
                                    out[dt_ * 128:(dt_ + 1) * 128, :],
                                    ot[:])

                    partA_sb = p2c.tile([128, KT, 512], f32, tag="partA")
                    with tc.tile_pool(name="ps_s", bufs=5,
                                      space="PSUM") as ps_s, \
                         tc.tile_pool(name="ps_pv", bufs=3,
                                      space="PSUM") as ps_pv:
                        for b in range(B):
                            for cl in range(TQC):
                                attention_chunk(0, b, cl, ps_s, ps_pv)
                        nc.gpsimd.collective_compute(
                            "AllToAll", mybir.AluOpType.bypass,
                            replica_groups=[list(range(NCORES))],
                            ins=[a2a_in[0].opt()], outs=[a2a_out[0].opt()])
                        for b in range(B):
                            for cl in range(TQC):
                                attention_chunk(1, b, cl, ps_s, ps_pv)
                    att2A = load_att2(0)
                    # A2A#2 runs on the collective engine while PE does oproj0
                    nc.gpsimd.collective_compute(
                        "AllToAll", mybir.AluOpType.bypass,
                        replica_groups=[list(range(NCORES))],
                        ins=[a2a_in[1].opt()], outs=[a2a_out[1].opt()])
                    with tc.tile_pool(name="ps_o", bufs=5,
                                      space="PSUM") as ps_o:
                        for g4 in range(3):
                            oproj_group(0, g4, att2A, partA_sb, ps_o)
                        att2B = load_att2(1)
                        for g4 in range(3):
                            oproj_group(1, g4, att2B, partA_sb, ps_o)

    nc.compile()
    return nc


def _plane_major(w):
    """Reorder head-dim rows 3k+i -> 32i+k (per 96-row head block)."""
    idx = np.empty(96, dtype=np.int64)
    for i in range(3):
        for k in range(NTRIP):
            idx[32 * i + k] = 3 * k + i
    return w[idx]


def _prep_inputs(x, w_qkv, w_o, Rs):
    x = np.asarray(x, dtype=np.float32)
    w_qkv = np.asarray(w_qkv, dtype=np.float32)
    w_o = np.asarray(w_o, dtype=np.float32)
    Rs = np.asarray(Rs, dtype=np.float32)

    xT = np.ascontiguousarray(x.reshape(NTOK, D_MODEL).T)

    # rope coefficients, plane-major rows: C[d, delta, t]
    R = Rs[:T]                                   # (T, 32, 3, 3)
    cco = np.empty((96, 3, T), dtype=np.float32)
    for d in range(3):
        for i in range(3):
            cco[32 * i:32 * i + 32, d, :] = R[:, :, i, (i + d) % 3].T

    # causal masks for the 4 diagonal sub-tiles
    msk = np.empty((128, 4, 512), dtype=np.float32)
    j = np.arange(128)[:, None]
    i = np.arange(512)[None, :]
    for m in range(4):
        msk[:, m, :] = (m * 128 + j <= i).astype(np.float32)

    # w_o columns-for-even-heads first, then odd (matches split A2A halves)
    woT = np.ascontiguousarray(w_o.T)            # rows e = h*96+d
    perm = np.concatenate(
        [np.arange(h * 96, (h + 1) * 96) for h in range(0, 16, 2)] +
        [np.arange(h * 96, (h + 1) * 96) for h in range(1, 16, 2)])
    woTp = np.ascontiguousarray(woT[perm])

    def w_row(s, h):                             # rows of w_qkv for (q/k/v, head)
        base = (s * N_HEADS + h) * HEAD_DIM
        return w_qkv[base:base + HEAD_DIM]

    in_maps = []
    for c in range(NCORES):
        h0, h1 = 2 * c, 2 * c + 1
        wall = np.concatenate([
            _plane_major(w_row(0, h0)) * SCALE,
            _plane_major(w_row(0, h1)) * SCALE,
            _plane_major(w_row(1, h0)),
            _plane_major(w_row(1, h1)),
            w_row(2, h0),
            w_row(2, h1),
        ], axis=0)                               # [576, 1536]
        wallT = np.ascontiguousarray(wall.T)     # [1536, 576]
        in_maps.append({
            "xT": xT, "wallT": wallT, "woT": woTp,
            "cco": cco, "msk": msk, "idn": np.eye(96, dtype=np.float32),
        })
    return in_maps


def kernel(x, w_qkv, w_o, Rs):
    from concourse import bass_utils

    if "nc" not in _CACHE:
        _CACHE["nc"] = _build_nc()
    nc = _CACHE["nc"]
    in_maps = _prep_inputs(x, w_qkv, w_o, Rs)
    res = bass_utils.run_bass_kernel_spmd(
        nc, in_maps, core_ids=list(range(NCORES)))
    full_T = np.concatenate([res.results[c]["out"] for c in range(NCORES)],
                            axis=1)              # [1536, 4096]
    return np.ascontiguousarray(full_T.T).reshape(B, T, D_MODEL)


# revision 17
# speedup vs baseline: 1.4295x; 1.0901x over previous
"""Trainium2 Bass kernel: causal attention with 3D (Rodrigues) RoPE.

Sharding: tensor-parallel over heads (2 heads/core on 8 cores) for
QKV projection + RoPE + SDPA, then an AllToAll redistributes attention
outputs so the output projection is sharded over tokens (512/core).
The A2A is split per local head so the first half overlaps attention,
and the output projection's contraction is split to match.

Layouts (per core, all matmuls in float32r):
  x^T       [1536, 4096]   tokens on the free axis
  qkv proj  computed transposed: out rows [q0 q1 k0 k1 v0 v1] (576) in
                           5 M-tiles of N=512; straddled tiles evicted
                           with 32-aligned cross-base copies on ScalarE
  q^T,k^T   [96, 4096]     head-dim on partitions (plane-major triplet
                           order so RoPE shifts are 32-row blocks)
  V         [4096, 194]    built by PE transposes of v^T; cols =
                           v_h0|1|v_h1|1 (the 1-columns give the softmax
                           denominator as row 96 of the PV matmul)
  S^T       [tk=128, tq=512] softmax's reduction axis = PE contraction
                           axis -> no transposes in attention.
"""

import sys

sys.path.insert(0, "/opt/trn_rl_repo")

import numpy as np

D_MODEL, N_HEADS, HEAD_DIM, MAX_POS = 1536, 16, 96, 4096
B, T = 2, 2048
NTOK = B * T                      # 4096
NCORES = 8
HPC = N_HEADS // NCORES           # 2 heads per core
NTRIP = HEAD_DIM // 3             # 32 triplets
KT = D_MODEL // 128               # 12 contraction tiles
NCH = NTOK // 512                 # 8 token chunks of 512
TQC = T // 512                    # 4 query chunks per batch
SCALE = 1.0 / np.sqrt(HEAD_DIM)

_CACHE = {}


def _build_nc():
    import concourse.bass as bass
    import concourse.mybir as mybir
    import concourse.tile as tile
    from concourse import bacc

    f32 = mybir.dt.float32
    f32r = mybir.dt.float32r
    MUL = mybir.AluOpType.mult
    ADD = mybir.AluOpType.add

    nc = bacc.Bacc("TRN2", target_bir_lowering=False, debug=False,
                   enable_asserts=False, num_devices=NCORES)

    xT = nc.dram_tensor("xT", [D_MODEL, NTOK], f32r, kind="ExternalInput").ap()
    wallT = nc.dram_tensor("wallT", [D_MODEL, 576], f32r, kind="ExternalInput").ap()
    woT = nc.dram_tensor("woT", [D_MODEL, D_MODEL], f32r, kind="ExternalInput").ap()
    cco = nc.dram_tensor("cco", [96, 3, T], f32, kind="ExternalInput").ap()
    msk = nc.dram_tensor("msk", [128, 128], f32r, kind="ExternalInput").ap()
    idn = nc.dram_tensor("idn", [96, 96], f32r, kind="ExternalInput").ap()
    out = nc.dram_tensor("out", [D_MODEL, 512], f32, kind="ExternalOutput").ap()

    with tile.TileContext(nc) as tc:
        with tc.tile_pool(name="dram", bufs=1, space="DRAM") as dram:
            a2a_in = [dram.tile([NCH, 96, 512], f32r, name=f"a2a_in{h}")
                      for h in range(HPC)]
            a2a_out = [dram.tile([NCH, 96, 512], f32r, name=f"a2a_out{h}")
                       for h in range(HPC)]

            with tc.tile_pool(name="ph12", bufs=1) as pp:
                qk_rot = [pp.tile([96, NTOK], f32r, tag=f"qkrot{i}",
                                  name=f"qkrot{i}") for i in range(4)]
                v_sb = pp.tile([128, NTOK // 128, 194], f32r, tag="vsb")

                # ------------ phase 1: qkv projection + rope ------------
                with tc.tile_pool(name="ph1", bufs=1) as p1, \
                     tc.tile_pool(name="ph1s", bufs=2) as p1s, \
                     tc.tile_pool(name="ps_qk", bufs=6, space="PSUM") as ps_qk, \
                     tc.tile_pool(name="ps_t", bufs=2, space="PSUM") as ps_t:
                    wall_sb = p1.tile([128, KT, 576], f32r, tag="wall")
                    nc.sync.dma_start(
                        wall_sb[:], wallT.rearrange("(k p) c -> p k c", p=128))
                    ident = p1.tile([96, 96], f32r, tag="ident")
                    nc.sync.dma_start(ident[:], idn[:])

                    prev_vr = [None]
                    CP = mybir.ActivationFunctionType.Copy

                    def emit_transposes(chp, vrp):
                        for hh in range(HPC):
                            for ts_ in range(4):
                                pt_ = ps_t.tile([128, 96], f32r, tag="tr",
                                                name="pt_")
                                nc.tensor.transpose(
                                    pt_[:],
                                    vrp[hh][:, ts_ * 128:(ts_ + 1) * 128],
                                    ident[:])
                                g = chp * 4 + ts_
                                nc.scalar.activation(
                                    v_sb[:, g, hh * 97:hh * 97 + 96],
                                    pt_[:], CP)
                                nc.vector.memset(
                                    v_sb[:, g, hh * 97 + 96:hh * 97 + 97]
                                    .bitcast(f32), 1.0)

                    for ch in range(NCH):
                        coff = (ch % TQC) * 512   # position within batch
                        c_sl = p1s.tile([96, 3, 512], f32, tag="csl")
                        nc.sync.dma_start(c_sl[:], cco[:, :, coff:coff + 512])
                        xt = []
                        for kt in range(KT):
                            t = p1s.tile([128, 512], f32r, tag=f"xt{kt}",
                                         name=f"xt{kt}")
                            nc.sync.dma_start(
                                t[:], xT[kt * 128:(kt + 1) * 128,
                                         ch * 512:(ch + 1) * 512])
                            xt.append(t)
                        # 5 M-tiles over rows [q0 q1 k0 k1 v0 v1] (576)
                        pst = []
                        for m in range(5):
                            mw = 128 if m < 4 else 64
                            ps = ps_qk.tile([128, 512], f32, tag="qk",
                                            name="ps")
                            for kt in range(KT):
                                nc.tensor.matmul(
                                    ps[0:mw, :],
                                    wall_sb[:, kt, m * 128:m * 128 + mw],
                                    xt[kt][:], start=(kt == 0),
                                    stop=(kt == KT - 1))
                            pst.append(ps)
                        # straddled eviction (ScalarE, 32-aligned pieces)
                        raws = []
                        for nm in ("q0", "q1", "k0", "k1"):
                            r = p1s.tile([96, 512], f32, tag=f"raw{nm}",
                                         name=f"raw{nm}")
                            raws.append(r)
                        vr = []
                        for hh in range(HPC):
                            r = p1s.tile([96, 512], f32r, tag=f"vr{hh}",
                                         name=f"vr{hh}")
                            vr.append(r)
                        ev = [
                            (raws[0][0:96, :], pst[0][0:96, :]),
                            (raws[1][0:32, :], pst[0][96:128, :]),
                            (raws[1][32:64, :], pst[1][0:32, :]),
                            (raws[1][64:96, :], pst[1][32:64, :]),
                            (raws[2][0:64, :], pst[1][64:128, :]),
                            (raws[2][64:96, :], pst[2][0:32, :]),
                            (raws[3][0:32, :], pst[2][32:64, :]),
                            (raws[3][32:64, :], pst[2][64:96, :]),
                            (raws[3][64:96, :], pst[2][96:128, :]),
                            (vr[0][0:96, :], pst[3][0:96, :]),
                            (vr[1][0:32, :], pst[3][96:128, :]),
                            (vr[1][32:64, :], pst[4][0:32, :]),
                            (vr[1][64:96, :], pst[4][32:64, :]),
                        ]
                        for ei, (dst_ap, src_ap) in enumerate(ev):
                            if ei % 2 == 0:
                                nc.scalar.activation(dst_ap, src_ap, CP)
                            else:
                                nc.vector.tensor_copy(dst_ap, src_ap)
                        # rope on q0,q1,k0,k1
                        for m in range(4):
                            raw = raws[m]
                            dst = qk_rot[m][:, ch * 512:(ch + 1) * 512]
                            nc.vector.tensor_tensor(
                                dst, raw[:], c_sl[:, 0, :], MUL)
                            g1 = p1s.tile([96, 512], f32, tag="g1")
                            nc.gpsimd.dma_start(g1[0:64, :], raw[32:96, :])
                            nc.gpsimd.dma_start(g1[64:96, :], raw[0:32, :])
                            g2 = p1s.tile([96, 512], f32, tag="g2")
                            nc.gpsimd.dma_start(g2[0:32, :], raw[64:96, :])
                            nc.gpsimd.dma_start(g2[32:96, :], raw[0:64, :])
                            nc.vector.tensor_tensor(
                                g1[:], g1[:], c_sl[:, 1, :], MUL)
                            nc.vector.tensor_tensor(dst, dst, g1[:], ADD)
                            nc.vector.tensor_tensor(
                                g2[:], g2[:], c_sl[:, 2, :], MUL)
                            nc.vector.tensor_tensor(dst, dst, g2[:], ADD)
                        # V: PE-transpose v^T of the PREVIOUS chunk so the
                        # transposes interleave with this chunk's projections
                        if ch > 0:
                            emit_transposes(ch - 1, prev_vr[0])
                        prev_vr[0] = vr
                    emit_transposes(NCH - 1, prev_vr[0])

                # ------------ phase 2: attention (+ overlapped A2A/o-proj) --
                with tc.tile_pool(name="ph2", bufs=6) as p2, \
                     tc.tile_pool(name="ph2b", bufs=3) as p2b, \
                     tc.tile_pool(name="ph2c", bufs=1) as p2c:
                    m_sb = p2c.tile([128, 128], f32r, tag="msb")
                    nc.sync.dma_start(m_sb[:], msk[:])

                    def attention_chunk(h, b, cl, ps_s, ps_pv):
                                qoff = b * T + cl * 512
                                pv = ps_pv.tile([128, 512], f32, tag="pv",
                                                name="pv")
                                ntk = 4 * cl + 4
                                order = (list(range(4 * cl, ntk)) +
                                         list(range(4 * cl)))
                                pend = []      # (ti, tt, lo, pt) awaiting PV

                                def emit_pv():
                                    ti, tt, lo, pt = pend.pop(0)
                                    nc.tensor.matmul(
                                        pv[0:97, lo:512],
                                        v_sb[:, b * 16 + tt,
                                             h * 97:h * 97 + 97],
                                        pt[:, lo:512], start=(ti == 0),
                                        stop=(ti == ntk - 1),
                                        skip_group_check=True)

                                for ti, tt in enumerate(order):
                                    koff = b * T + tt * 128
                                    # diagonal tiles: columns < lo are fully
                                    # masked -> skip them in QK/exp/PV
                                    lo = (tt - 4 * cl) * 128 if tt >= 4 * cl \
                                        else 0
                                    sp = ps_s.tile([128, 512], f32, tag="s",
                                                   name="sp")
                                    nc.tensor.matmul(
                                        sp[:, lo:512],
                                        qk_rot[2 + h][:, koff:koff + 128],
                                        qk_rot[h][:, qoff + lo:qoff + 512],
                                        start=True, stop=True)
                                    pt = p2.tile([128, 512], f32r, tag="p",
                                                 name="pt")
                                    nc.scalar.activation(
                                        pt[:, lo:512], sp[:, lo:512],
                                        mybir.ActivationFunctionType.Exp)
                                    if tt >= 4 * cl:
                                        nc.vector.tensor_tensor(
                                            pt[:, lo:lo + 128],
                                            pt[:, lo:lo + 128],
                                            m_sb[:], MUL)
                                    pend.append((ti, tt, lo, pt))
                                    if len(pend) > 2:
                                        emit_pv()
                                while pend:
                                    emit_pv()
                                linv = p2b.tile([1, 512], f32, tag="linv",
                                                name="linv")
                                nc.vector.reciprocal(linv[:], pv[96:97, :])
                                brow = p2b.tile([96, 512], f32, tag="brow",
                                                name="brow")
                                nc.gpsimd.partition_broadcast(brow[:], linv[:])
                                att = p2b.tile([96, 512], f32r, tag="att",
                                               name="att")
                                nc.vector.tensor_tensor(
                                    att[:], pv[0:96, :], brow[:], MUL)
                                return nc.sync.dma_start(
                                    a2a_in[h][b * TQC + cl, :, :], att[:])

                    def load_att2(half, after=None):
                        flat = a2a_out[half][:].rearrange("a b c -> (a b) c")
                        att2 = []
                        for et in range(6):
                            t = p2c.tile([128, 512], f32r,
                                         tag=f"att2_{half}_{et}",
                                         name=f"att2_{half}_{et}")
                            ld = nc.sync.dma_start(
                                t[:], flat[et * 128:(et + 1) * 128, :])
                            if after is not None:
                                tile.add_dep_helper(
                                    ld.ins, after.ins, sync=False,
                                    reason="sync queue order")
                            att2.append(t)
                        return att2

                    def oproj_group(half, g4, att2, partA_sb, ps_o):
                        """one group of 4 dt tiles x full 6-K-tile half."""
                        pos = [ps_o.tile([128, 512], f32, tag="o",
                                         name=f"po_{half}_{g4}_{i}")
                               for i in range(4)]
                        for et in range(6):
                            wot = p2c.tile([128, 512], f32r, tag="wos",
                                           bufs=4, name="wot")
                            nc.sync.dma_start(
                                wot[:],
                                woT[half * 768 + et * 128:
                                    half * 768 + (et + 1) * 128,
                                    g4 * 512:(g4 + 1) * 512])
                            for i in range(4):
                                nc.tensor.matmul(
                                    pos[i][:],
                                    wot[:, i * 128:(i + 1) * 128],
                                    att2[et][:], start=(et == 0),
                                    stop=(et == 5),
                                    skip_group_check=True)
                        for i in range(4):
                            dt_ = g4 * 4 + i
                            if half == 0:
                                nc.vector.tensor_copy(
                                    partA_sb[:, dt_, :], pos[i][:])
                            else:
                                ot = p2b.tile([128, 512], f32, tag="ot",
                                              bufs=3, name="ot")
                                nc.vector.tensor_tensor(
                                    ot[:], pos[i][:],
                                    partA_sb[:, dt_, :], ADD)
                                nc.sync.dma_start(
                                    out[dt_ * 128:(dt_ + 1) * 128, :],
                                    ot[:])

                    partA_sb = p2c.tile([128, KT, 512], f32, tag="partA")
                    with tc.tile_pool(name="ps_s", bufs=5,
                                      space="PSUM") as ps_s, \
                         tc.tile_pool(name="ps_pv", bufs=3,
                                      space="PSUM") as ps_pv:
                        for b in range(B):
                            for cl in range(TQC):
                                attention_chunk(0, b, cl, ps_s, ps_pv)
                        with tc.high_priority():
                            nc.gpsimd.collective_compute(
                                "AllToAll", mybir.AluOpType.bypass,
                                replica_groups=[list(range(NCORES))],
                                ins=[a2a_in[0].opt()], outs=[a2a_out[0].opt()])
                        last_w = None
                        for b in range(B):
                            for cl in range(TQC):
                                last_w = attention_chunk(1, b, cl, ps_s, ps_pv)
                    att2A = load_att2(0, last_w)
                    # A2A#2 runs on the collective engine while PE does oproj0
                    with tc.high_priority():
                        nc.gpsimd.collective_compute(
                            "AllToAll", mybir.AluOpType.bypass,
                            replica_groups=[list(range(NCORES))],
                            ins=[a2a_in[1].opt()], outs=[a2a_out[1].opt()])
                    with tc.tile_pool(name="ps_o", bufs=5,
                                      space="PSUM") as ps_o:
                        for g4 in range(3):
                            oproj_group(0, g4, att2A, partA_sb, ps_o)
                        att2B = load_att2(1)
                        for g4 in range(3):
                            oproj_group(1, g4, att2B, partA_sb, ps_o)

    nc.compile()
    return nc


def _plane_major(w):
    """Reorder head-dim rows 3k+i -> 32i+k (per 96-row head block)."""
    idx = np.empty(96, dtype=np.int64)
    for i in range(3):
        for k in range(NTRIP):
            idx[32 * i + k] = 3 * k + i
    return w[idx]


def _prep_inputs(x, w_qkv, w_o, Rs):
    x = np.asarray(x, dtype=np.float32)
    w_qkv = np.asarray(w_qkv, dtype=np.float32)
    w_o = np.asarray(w_o, dtype=np.float32)
    Rs = np.asarray(Rs, dtype=np.float32)

    xT = np.ascontiguousarray(x.reshape(NTOK, D_MODEL).T)

    # rope coefficients, plane-major rows: C[d, delta, t]
    R = Rs[:T]                                   # (T, 32, 3, 3)
    cco = np.empty((96, 3, T), dtype=np.float32)
    for d in range(3):
        for i in range(3):
            cco[32 * i:32 * i + 32, d, :] = R[:, :, i, (i + d) % 3].T

    # lower-triangular mask for the mixed 128x128 diagonal block
    j = np.arange(128)[:, None]
    i = np.arange(128)[None, :]
    msk = (j <= i).astype(np.float32)

    # w_o columns-for-even-heads first, then odd (matches split A2A halves)
    woT = np.ascontiguousarray(w_o.T)            # rows e = h*96+d
    perm = np.concatenate(
        [np.arange(h * 96, (h + 1) * 96) for h in range(0, 16, 2)] +
        [np.arange(h * 96, (h + 1) * 96) for h in range(1, 16, 2)])
    woTp = np.ascontiguousarray(woT[perm])

    def w_row(s, h):                             # rows of w_qkv for (q/k/v, head)
        base = (s * N_HEADS + h) * HEAD_DIM
        return w_qkv[base:base + HEAD_DIM]

    in_maps = []
    for c in range(NCORES):
        h0, h1 = 2 * c, 2 * c + 1
        wall = np.concatenate([
            _plane_major(w_row(0, h0)) * SCALE,
            _plane_major(w_row(0, h1)) * SCALE,
            _plane_major(w_row(1, h0)),
            _plane_major(w_row(1, h1)),
            w_row(2, h0),
            w_row(2, h1),
        ], axis=0)                               # [576, 1536]
        wallT = np.ascontiguousarray(wall.T)     # [1536, 576]
        in_maps.append({
            "xT": xT, "wallT": wallT, "woT": woTp,
            "cco": cco, "msk": msk, "idn": np.eye(96, dtype=np.float32),
        })
    return in_maps


def kernel(x, w_qkv, w_o, Rs):
    from concourse import bass_utils

    if "nc" not in _CACHE:
        _CACHE["nc"] = _build_nc()
    nc = _CACHE["nc"]
    in_maps = _prep_inputs(x, w_qkv, w_o, Rs)
    res = bass_utils.run_bass_kernel_spmd(
        nc, in_maps, core_ids=list(range(NCORES)))
    full_T = np.concatenate([res.results[c]["out"] for c in range(NCORES)],
                            axis=1)              # [1536, 4096]
    return np.ascontiguousarray(full_T.T).reshape(B, T, D_MODEL)
